# revision 6
# baseline (speedup 1.0000x reference)
"""Trainium2 Bass kernel for causal masked-ReLU attention (no softmax).

Reference computation (B=8, T=1024, C=768, n_head=12, hd=64):
    qkv = x @ W_attn.T + b_attn
    q, k, v = split(qkv); per-head: att = relu(mask_causal(q k^T / sqrt(hd)))
    y = att @ v, heads re-merged -> (B, T, C)

Sharding: one batch element per NeuronCore (8 cores). Each core computes the
QKV projection and all 12 heads' attention for its batch element.

Structure (per core) — single interleaved pipeline, PE-roofline oriented:
  - Projection output layouts: qkT[m] = [128 ch, T] (q/k head-pairs
    interleaved so heads 2a/2a+1 share a partition base), v_sb[t] =
    [128 kpos, 12*64 ch].
  - QK^T is computed as att.T tiles [128 kpos, q] (kept region only, at
    128-col causal granularity), evicted by DVE (masked diag piece via
    relu*[tri|1s]) and ACT (pure-relu piece).
  - AV streams v as the MOVING operand: y[q, d] += att.T[kpos, q].T @
    v[kpos, d]; output tiles [128 q, 64 d] accumulate over kpos tiles in
    PSUM.  This halves AV's PE streaming cost vs. streaming att, and means
    below-diagonal att regions are never read (no zero-fill needed).
  - Head-pair chunks are interleaved with the NEXT pair's qkv projection
    groups in the PE stream: [QK(a) tk0-3][AV(a) tq0-3][QK(a) tk4-7]
    [proj(a+1)][AV(a) tq4-7].  Eviction (ACT/DVE) demand per chunk sits
    well under the PE time of a chunk, so the PE never waits on evictions.
  - All matmuls use fp16 operands (1 PE cycle/row at any width), fp32 PSUM.
  - Inputs ship as fp16; q-side weights/bias pre-scaled by 1/sqrt(hd).
  - Input DMAs are chained and the v-projection phase runs k-outer over
    two 8-group windows so the PE starts while tiles land; dummy matmuls
    during the initial DMA wait keep the PE p-state ramp warm.
  - y is produced in natural (T, C) layout fp16; host just upcasts.
"""

import numpy as np

import sys
for _p in ("/opt/trn_rl_repo", "/root/.axon_site", "/root/.axon_site/_ro/trn_rl_repo",
           "/root/.axon_site/_ro/pypackages"):
    if _p not in sys.path:
        sys.path.append(_p)

import concourse.bacc as bacc
import concourse.mybir as mybir
from concourse.alu_op_type import AluOpType
from concourse.tile import TileContext
from concourse.tile_rust import add_dep_helper
from concourse.bass_utils import run_bass_kernel_spmd

B, T, C = 8, 1024, 768
NH, HD = 12, 64
C3 = 3 * C            # 2304
KT = C // 128         # 6  contraction tiles of the projection
TT = T // 128         # 8  tiles of the sequence dim
NPAIR = NH // 2       # 6  head pairs
F32 = mybir.dt.float32
F16 = mybir.dt.float16
AF = mybir.ActivationFunctionType

WARM_MMS = 8
STT_W = 256           # width of the DVE masked-relu diag piece per k-tile

_CACHE = {}


def _build():
    nc = bacc.Bacc("TRN2", target_bir_lowering=False, debug=False, num_devices=8)

    xT = nc.dram_tensor("xT", [C, T], F16, kind="ExternalInput").ap()
    wT = nc.dram_tensor("wT", [C, C3], F16, kind="ExternalInput").ap()
    bqk = nc.dram_tensor("bqk", [128, 2 * NPAIR], F32, kind="ExternalInput").ap()
    bvb = nc.dram_tensor("bvb", [128, C], F32, kind="ExternalInput").ap()
    # masks = [tri(128) | ones(384)]: the kept region of att.T tile tk always
    # starts with the triangular diagonal block, so masks[:, :width] is the
    # relu-mask for any kept piece up to 512 wide
    masks = nc.dram_tensor("masks", [128, 512], F32, kind="ExternalInput").ap()
    y = nc.dram_tensor("y", [T, C], F16, kind="ExternalOutput").ap()

    dma_chain = []
    CHAIN_DEPTH = 6

    def chained_dma(out, in_):
        inst = nc.sync.dma_start(out=out, in_=in_)
        if len(dma_chain) >= CHAIN_DEPTH:
            add_dep_helper(inst.ins, dma_chain[-CHAIN_DEPTH].ins, True,
                           "dma staging chain")
        dma_chain.append(inst)
        return inst

    with TileContext(nc) as tc:
        with (
            tc.tile_pool(name="persist", bufs=1) as pp,
            tc.tile_pool(name="psA", bufs=5, space="PSUM") as psA,
            tc.tile_pool(name="psB", bufs=3, space="PSUM") as psB,
        ):
            masks_sb = pp.tile([128, 512], F32, name="masks_sb")
            bqk_sb = pp.tile([128, 2 * NPAIR], F32, name="bqk_sb")
            bvb_sb = pp.tile([128, C], F32, name="bvb_sb")
            w_sb = [pp.tile([128, C3], F16, name=f"w{k}") for k in range(KT)]
            x_sb = [pp.tile([128, T], F16, name=f"x{k}") for k in range(KT)]
            qkT = [pp.tile([128, T], F16, name=f"qkT{m}") for m in range(2 * NPAIR)]
            v_sb = [pp.tile([128, C], F16, name=f"v{t}") for t in range(TT)]
            att = [[pp.tile([128, T], F16, name=f"att{s}_{t}")
                    for t in range(TT)] for s in range(2)]
            # y natural layout, tq-major chunks of C columns
            y_sb = pp.tile([128, TT * C], F16, name="y_sb")

            # ---------- input DMAs ----------
            # x + w-v first (v-phase streams them k-outer), then w-qk per k
            smalls = [(bvb_sb, bvb), (bqk_sb, bqk)]
            for k in range(KT):
                chained_dma(x_sb[k][:], xT[128 * k:128 * (k + 1), :])
                chained_dma(w_sb[k][:, 2 * C:],
                            wT[128 * k:128 * (k + 1), 2 * C:])
                if smalls and k >= 1:
                    dst, src_ = smalls.pop(0)
                    chained_dma(dst[:], src_[:])
            for k in range(KT):
                chained_dma(w_sb[k][:, :2 * C],
                            wT[128 * k:128 * (k + 1), :2 * C])
            chained_dma(masks_sb[:], masks[:])

            # ---------- PE warmup during initial DMA wait ----------
            scratch = pp.tile([128, 512], F16, name="warm_src")
            nc.vector.memset(scratch[:], 0.0)
            for _ in range(WARM_MMS):
                warm = psA.tile([128, 512], F32, name="ps_warm", tag="psA")
                nc.tensor.matmul(warm[:], scratch[:, :128], scratch[:],
                                 start=True, stop=True)

            # ---------- v projection: 2 windows of 8 groups, k-outer ----------
            vgroups = [(t, n0, nw) for t in range(TT)
                       for (n0, nw) in ((0, 512), (512, 256))]
            for w0 in (0, 8):
                window = vgroups[w0:w0 + 8]
                tiles = []
                for i, g in enumerate(window):
                    pool = psA if i < 5 else psB
                    tiles.append(pool.tile([128, 512], F32, name="ps_v",
                                           tag=("psA" if i < 5 else "psB")))
                for k in range(KT):
                    for (t, n0, nw), ps in zip(window, tiles):
                        nc.tensor.matmul(
                            ps[:, :nw],
                            x_sb[k][:, 128 * t:128 * (t + 1)],
                            w_sb[k][:, 2 * C + n0:2 * C + n0 + nw],
                            start=(k == 0), stop=(k == KT - 1),
                        )
                for (t, n0, nw), ps in zip(window, tiles):
                    nc.vector.tensor_tensor(
                        v_sb[t][:, n0:n0 + nw], ps[:, :nw],
                        bvb_sb[:, n0:n0 + nw], AluOpType.add,
                    )

            # ---------- helpers ----------
            def proj_pair(a):
                # qkv projection for m-tiles 2a (q pair) and 2a+1 (k pair)
                for m in (2 * a, 2 * a + 1):
                    for half in (0, 1):
                        ps = psA.tile([128, 512], F32, name="ps_qkp", tag="psA")
                        q0 = 512 * half
                        for k in range(KT):
                            nc.tensor.matmul(
                                ps[:],
                                w_sb[k][:, 128 * m:128 * (m + 1)],
                                x_sb[k][:, q0:q0 + 512],
                                start=(k == 0), stop=(k == KT - 1),
                            )
                        nc.vector.tensor_scalar(
                            qkT[m][:, q0:q0 + 512], ps[:],
                            bqk_sb[:, m:m + 1], None,
                            AluOpType.add,
                        )

            def qk_head(a, r, tks):
                # att.T tiles [128 kpos, kept q] for head 2a+r.  Piece layout
                # per tk: a narrow diag piece (DVE masked-relu) followed by
                # <=512-wide pure-relu pieces (ACT), sized to balance engines.
                qh = qkT[2 * a][64 * r:64 * (r + 1), :]
                kh = qkT[2 * a + 1][64 * r:64 * (r + 1), :]
                for tk in tks:
                    k0 = 128 * tk
                    w = T - k0
                    s = min(STT_W, w)
                    # piece 1: diag, masked relu on DVE
                    ps = psA.tile([128, 512], F32, name="ps_qk", tag="psA")
                    nc.tensor.matmul(ps[:, :s], kh[:, k0:k0 + 128],
                                     qh[:, k0:k0 + s], start=True, stop=True)
                    nc.vector.scalar_tensor_tensor(
                        att[r][tk][:, k0:k0 + s], ps[:, :s],
                        0.0, masks_sb[:, :s],
                        AluOpType.max, AluOpType.mult,
                    )
                    # remaining pieces: pure relu on ACT
                    q0 = k0 + s
                    while q0 < T:
                        pw = min(512, T - q0)
                        ps2 = psA.tile([128, 512], F32, name="ps_qk", tag="psA")
                        nc.tensor.matmul(ps2[:, :pw], kh[:, k0:k0 + 128],
                                         qh[:, q0:q0 + pw], start=True, stop=True)
                        nc.scalar.activation(att[r][tk][:, q0:q0 + pw],
                                             ps2[:, :pw], AF.Relu)
                        q0 += pw

            def av_part(a, tqs):
                # y[q, d] accumulation for pair a over q-tiles tqs.
                # psum half-tile layout: [tq-block: r0 64 | r1 64] x 4
                psh = psB.tile([128, 512], F32, name="ps_av", tag="psB")
                for tq in tqs:
                    for r in (0, 1):
                        h = 2 * a + r
                        col = 128 * (tq % 4) + 64 * r
                        for tk in range(tq + 1):
                            nc.tensor.matmul(
                                psh[:, col:col + 64],
                                att[r][tk][:, 128 * tq:128 * (tq + 1)],
                                v_sb[tk][:, 64 * h:64 * (h + 1)],
                                start=(tk == 0), stop=(tk == tq),
                            )
                return psh

            y3 = y_sb[:].rearrange("p (t c) -> p t c", t=TT)

            def av_evict(a, tqs, psh, engine):
                # one strided op: 4 tq-blocks of 128 cols -> y_sb chunks
                p3 = psh[:].rearrange("p (t c) -> p t c", t=4)
                dst = y3[:, tqs[0]:tqs[0] + 4, 128 * a:128 * (a + 1)]
                if engine == "act":
                    nc.scalar.copy(dst, p3)
                else:
                    nc.vector.tensor_scalar(dst, p3, 0.0, None, AluOpType.add)

            # ---------- interleaved pair chunks ----------
            # chunk(a) = [QK(a) all tks] [proj(a+1)] [AV(a)]: evictions of
            # QK(a) drain during the proj window before AV(a) consumes them.
            proj_pair(0)
            for a in range(NPAIR):
                qk_head(a, 0, range(0, 8))
                qk_head(a, 1, range(0, 8))
                if a + 1 < NPAIR:
                    proj_pair(a + 1)
                if a < NPAIR - 1:
                    psh1 = av_part(a, (0, 1, 2, 3))
                    av_evict(a, (0, 1, 2, 3), psh1, "act")
                    psh2 = av_part(a, (4, 5, 6, 7))
                    av_evict(a, (4, 5, 6, 7), psh2, "act")
                else:
                    # finale: per-tq evict + y DMA as soon as each chunk
                    # completes, to shorten the tail
                    for half, tqs in ((0, (0, 1, 2, 3)), (1, (4, 5, 6, 7))):
                        psh = psB.tile([128, 512], F32, name="ps_av", tag="psB")
                        for tq in tqs:
                            for r in (0, 1):
                                h = 2 * a + r
                                col = 128 * (tq % 4) + 64 * r
                                for tk in range(tq + 1):
                                    nc.tensor.matmul(
                                        psh[:, col:col + 64],
                                        att[r][tk][:, 128 * tq:128 * (tq + 1)],
                                        v_sb[tk][:, 64 * h:64 * (h + 1)],
                                        start=(tk == 0), stop=(tk == tq),
                                    )
                            # evict just this tq's 128 cols and ship it
                            c0 = 128 * (tq % 4)
                            if tq % 2 == 0:
                                nc.scalar.copy(
                                    y3[:, tq, 128 * a:128 * (a + 1)],
                                    psh[:, c0:c0 + 128])
                            else:
                                nc.vector.tensor_scalar(
                                    y3[:, tq, 128 * a:128 * (a + 1)],
                                    psh[:, c0:c0 + 128], 0.0, None,
                                    AluOpType.add)
                            nc.sync.dma_start(
                                out=y[128 * tq:128 * (tq + 1), :],
                                in_=y3[:, tq, :])

    nc.compile()
    return nc


def _prep_host(x, W_attn, b_attn):
    s = 1.0 / np.sqrt(np.float32(HD))
    W = np.asarray(W_attn, dtype=np.float32).copy()
    b = np.asarray(b_attn, dtype=np.float32).copy()
    W[:C] *= s
    b[:C] *= s
    # interleave q/k head pairs: [q-pair0, k-pair0, q-pair1, k-pair1, ...], v natural
    rows = []
    for a in range(NPAIR):
        rows.extend(range(128 * a, 128 * (a + 1)))          # q heads 2a, 2a+1
        rows.extend(range(C + 128 * a, C + 128 * (a + 1)))  # k heads 2a, 2a+1
    rows.extend(range(2 * C, 3 * C))                        # v natural
    W_perm = W[rows]
    b_perm = b[rows]

    wT = np.ascontiguousarray(W_perm.T.astype(np.float16))   # (C, 3C)
    bqk = np.ascontiguousarray(b_perm[:2 * C].reshape(2 * NPAIR, 128).T)  # (128, 12)
    bvb = np.ascontiguousarray(np.broadcast_to(b_perm[2 * C:], (128, C)))
    tri = (np.arange(128)[None, :] >= np.arange(128)[:, None]).astype(np.float32)
    masks = np.ones((128, 512), dtype=np.float32)
    masks[:, 0:128] = tri          # kept pieces always start at the diagonal
    xT = np.ascontiguousarray(np.asarray(x, dtype=np.float32).transpose(0, 2, 1).astype(np.float16))  # (B, C, T)
    return xT, wT, bqk, bvb, masks


def kernel(x, W_attn, b_attn):
    if "nc" not in _CACHE:
        _CACHE["nc"] = _build()
    nc = _CACHE["nc"]

    xT, wT, bqk, bvb, masks = _prep_host(x, W_attn, b_attn)
    in_maps = [
        {"xT": xT[c], "wT": wT, "bqk": bqk, "bvb": bvb, "masks": masks}
        for c in range(B)
    ]
    res = run_bass_kernel_spmd(nc, in_maps, list(range(B)))
    yout = np.empty((B, T, C), dtype=np.float32)
    for c in range(B):
        yout[c] = res.results[c]["y"].astype(np.float32)
    return yout


# revision 28
# speedup vs baseline: 1.2304x; 1.2304x over previous
"""Trainium2 Bass kernel for causal masked-ReLU attention (no softmax).

Reference computation (B=8, T=1024, C=768, n_head=12, hd=64):
    qkv = x @ W_attn.T + b_attn
    q, k, v = split(qkv); per-head: att = relu(mask_causal(q k^T / sqrt(hd)))
    y = att @ v, heads re-merged -> (B, T, C)

Sharding: one batch element per NeuronCore (8 cores). Each core computes the
QKV projection and all 12 heads' attention for its batch element.

Structure (per core) — single interleaved pipeline, PE-roofline oriented:
  - Projection output layouts: qkT[m] = [128 ch, T] (q/k head-pairs
    interleaved so heads 2a/2a+1 share a partition base), v_sb[t] =
    [128 kpos, 12*64 ch].
  - QK^T is computed as att.T tiles [128 kpos, q] (kept region only, at
    128-col causal granularity), evicted by DVE (masked diag piece via
    relu*[tri|1s]) and ACT (pure-relu piece).
  - AV streams v as the MOVING operand: y[q, d] += att.T[kpos, q].T @
    v[kpos, d]; output tiles [128 q, 64 d] accumulate over kpos tiles in
    PSUM.  This halves AV's PE streaming cost vs. streaming att, and means
    below-diagonal att regions are never read (no zero-fill needed).
  - Head-pair chunks are interleaved with the NEXT pair's qkv projection
    groups in the PE stream: [QK(a) tk0-3][AV(a) tq0-3][QK(a) tk4-7]
    [proj(a+1)][AV(a) tq4-7].  Eviction (ACT/DVE) demand per chunk sits
    well under the PE time of a chunk, so the PE never waits on evictions.
  - All matmuls use fp16 operands (1 PE cycle/row at any width), fp32 PSUM.
  - Inputs ship as fp16; q-side weights/bias pre-scaled by 1/sqrt(hd).
  - Input DMAs are chained and the v-projection phase runs k-outer over
    two 8-group windows so the PE starts while tiles land; dummy matmuls
    during the initial DMA wait keep the PE p-state ramp warm.
  - y is produced in natural (T, C) layout fp16; host just upcasts.
"""

import numpy as np

import sys
for _p in ("/opt/trn_rl_repo", "/root/.axon_site", "/root/.axon_site/_ro/trn_rl_repo",
           "/root/.axon_site/_ro/pypackages"):
    if _p not in sys.path:
        sys.path.append(_p)

import concourse.bacc as bacc
import concourse.mybir as mybir
from concourse.alu_op_type import AluOpType
from concourse.tile import TileContext
from concourse.tile_rust import add_dep_helper
from concourse.bass_utils import run_bass_kernel_spmd

B, T, C = 8, 1024, 768
NH, HD = 12, 64
C3 = 3 * C            # 2304
KT = C // 128         # 6  contraction tiles of the projection
TT = T // 128         # 8  tiles of the sequence dim
NPAIR = NH // 2       # 6  head pairs
F32 = mybir.dt.float32
F16 = mybir.dt.float16
AF = mybir.ActivationFunctionType

WARM_MMS = 2
STT_W = 512           # width of the DVE masked-relu diag piece per k-tile

_CACHE = {}


def _build():
    nc = bacc.Bacc("TRN2", target_bir_lowering=False, debug=False, num_devices=8)

    xT = nc.dram_tensor("xT", [C, T], F16, kind="ExternalInput").ap()
    wT = nc.dram_tensor("wT", [C, C3], F16, kind="ExternalInput").ap()
    bqk = nc.dram_tensor("bqk", [128, 2 * NPAIR], F32, kind="ExternalInput").ap()
    bvb = nc.dram_tensor("bvb", [128, C], F32, kind="ExternalInput").ap()
    # masks = [tri(128) | ones(384)]: the kept region of att.T tile tk always
    # starts with the triangular diagonal block, so masks[:, :width] is the
    # relu-mask for any kept piece up to 512 wide
    masks = nc.dram_tensor("masks", [128, 512], F32, kind="ExternalInput").ap()
    y = nc.dram_tensor("y", [T, C], F16, kind="ExternalOutput").ap()

    dma_chain = []
    CHAIN_DEPTH = 6

    def chained_dma(out, in_):
        inst = nc.sync.dma_start(out=out, in_=in_)
        if len(dma_chain) >= CHAIN_DEPTH:
            add_dep_helper(inst.ins, dma_chain[-CHAIN_DEPTH].ins, True,
                           "dma staging chain")
        dma_chain.append(inst)
        return inst

    with TileContext(nc) as tc:
        with (
            tc.tile_pool(name="persist", bufs=1) as pp,
            tc.tile_pool(name="psA", bufs=4, space="PSUM") as psA,
            tc.tile_pool(name="psP", bufs=2, space="PSUM") as psP,
            tc.tile_pool(name="psB", bufs=2, space="PSUM") as psB,
        ):
            masks_sb = pp.tile([128, 512], F32, name="masks_sb")
            bqk_sb = pp.tile([128, 2 * NPAIR], F32, name="bqk_sb")
            bvb_sb = pp.tile([128, C], F32, name="bvb_sb")
            w_sb = [pp.tile([128, C3], F16, name=f"w{k}") for k in range(KT)]
            x_sb = [pp.tile([128, T], F16, name=f"x{k}") for k in range(KT)]
            qkT = [pp.tile([128, T], F16, name=f"qkT{m}") for m in range(2 * NPAIR)]
            v_sb = [pp.tile([128, C], F16, name=f"v{t}") for t in range(TT)]
            # 6 att sets: pair a uses sets 2*(a%3), 2*(a%3)+1 — deep enough
            # that QK(a+1) runs a full stage ahead of AV(a) without WAR stalls
            att = [[pp.tile([128, T], F16, name=f"att{s}_{t}")
                    for t in range(TT)] for s in range(6)]
            # y staging: one tile per head-pair, [128 q-part, 8 tq x 128 cols];
            # each pair's strip DMAs to DRAM independently (no cross-pair WAR)
            ya = [pp.tile([128, TT * 128], F16, name=f"ya{a}")
                  for a in range(NPAIR)]

            # ---------- input DMAs ----------
            # x + w-v first (v-phase streams them k-outer), then w-qk per k
            smalls = [(bvb_sb, bvb), (bqk_sb, bqk)]
            for k in range(KT):
                chained_dma(x_sb[k][:], xT[128 * k:128 * (k + 1), :])
                chained_dma(w_sb[k][:, 2 * C:],
                            wT[128 * k:128 * (k + 1), 2 * C:])
                if smalls and k >= 1:
                    dst, src_ = smalls.pop(0)
                    chained_dma(dst[:], src_[:])
            for k in range(KT):
                chained_dma(w_sb[k][:, :2 * C],
                            wT[128 * k:128 * (k + 1), :2 * C])
            chained_dma(masks_sb[:], masks[:])

            # ---------- PE warmup during initial DMA wait ----------
            scratch = pp.tile([128, 512], F16, name="warm_src")
            nc.vector.memset(scratch[:], 0.0)
            for _ in range(WARM_MMS):
                warm = psA.tile([128, 512], F32, name="ps_warm", tag="psA")
                nc.tensor.matmul(warm[:], scratch[:, :128], scratch[:],
                                 start=True, stop=True)

            # ---------- v projection: 2 windows of 8 groups, k-outer ----------
            vgroups = [(t, n0, nw) for t in range(TT)
                       for (n0, nw) in ((0, 512), (512, 256))]

            def v_evict(g, ps):
                t, n0, nw = g
                nc.vector.tensor_tensor(
                    v_sb[t][:, n0:n0 + nw], ps[:, :nw],
                    bvb_sb[:, n0:n0 + nw], AluOpType.add,
                )

            # window 1: first 8 v groups k-outer (overlaps the input DMA)
            window = vgroups[:8]
            tiles = []
            for i, g in enumerate(window):
                pool, tg = ((psA, "psA") if i < 4 else
                            (psP, "psP") if i < 6 else (psB, "psB"))
                tiles.append(pool.tile([128, 512], F32, name="ps_v", tag=tg))
            for k in range(KT):
                for (t, n0, nw), ps in zip(window, tiles):
                    nc.tensor.matmul(
                        ps[:, :nw],
                        x_sb[k][:, 128 * t:128 * (t + 1)],
                        w_sb[k][:, 2 * C + n0:2 * C + n0 + nw],
                        start=(k == 0), stop=(k == KT - 1),
                    )
            for g, ps in zip(window, tiles):
                v_evict(g, ps)

            # remaining v groups: group-major (evictions spread out),
            # interleaved with pair-0's projection groups
            def v_group(g, pool, tg):
                t, n0, nw = g
                ps = pool.tile([128, 512], F32, name="ps_v", tag=tg)
                for k in range(KT):
                    nc.tensor.matmul(
                        ps[:, :nw],
                        x_sb[k][:, 128 * t:128 * (t + 1)],
                        w_sb[k][:, 2 * C + n0:2 * C + n0 + nw],
                        start=(k == 0), stop=(k == KT - 1),
                    )
                v_evict(g, ps)

            # ---------- helpers ----------
            def proj_group(a, g):
                # one qkv projection group: m-tile 2a + g//2, T-half g%2
                m = 2 * a + g // 2
                q0 = 512 * (g % 2)
                ps = psP.tile([128, 512], F32, name="ps_qkp", tag="psP")
                for k in range(KT):
                    nc.tensor.matmul(
                        ps[:],
                        w_sb[k][:, 128 * m:128 * (m + 1)],
                        x_sb[k][:, q0:q0 + 512],
                        start=(k == 0), stop=(k == KT - 1),
                    )
                nc.scalar.activation(
                    qkT[m][:, q0:q0 + 512], ps[:],
                    AF.Identity, bias=bqk_sb[:, m:m + 1], scale=1.0,
                )

            def attset(a, r):
                return att[2 * (a % 3) + r]

            def qk_tk(a, r, tk):
                # one att.T tile [128 kpos, kept q] for head 2a+r.  Piece
                # layout: a diag piece (DVE masked-relu) followed by a
                # <=512-wide pure-relu piece (ACT).
                qh = qkT[2 * a][64 * r:64 * (r + 1), :]
                kh = qkT[2 * a + 1][64 * r:64 * (r + 1), :]
                at = attset(a, r)
                k0 = 128 * tk
                w = T - k0
                s = min(STT_W, w)
                # piece 1: diag, masked relu on DVE
                ps = psA.tile([128, 512], F32, name="ps_qk", tag="psA")
                nc.tensor.matmul(ps[:, :s], kh[:, k0:k0 + 128],
                                 qh[:, k0:k0 + s], start=True, stop=True)
                nc.vector.scalar_tensor_tensor(
                    at[tk][:, k0:k0 + s], ps[:, :s],
                    0.0, masks_sb[:, :s],
                    AluOpType.max, AluOpType.mult,
                )
                # remaining pieces: pure relu on ACT
                q0 = k0 + s
                while q0 < T:
                    pw = min(512, T - q0)
                    ps2 = psA.tile([128, 512], F32, name="ps_qk", tag="psA")
                    nc.tensor.matmul(ps2[:, :pw], kh[:, k0:k0 + 128],
                                     qh[:, q0:q0 + pw], start=True, stop=True)
                    nc.scalar.activation(at[tk][:, q0:q0 + pw],
                                         ps2[:, :pw], AF.Relu)
                    q0 += pw

            def av_part(a, tqs):
                # y[q, d] accumulation for pair a over q-tiles tqs.
                # psum half-tile layout: [tq-block: r0 64 | r1 64] x 4
                psh = psB.tile([128, 512], F32, name="ps_av", tag="psB")
                for tq in tqs:
                    for r in (0, 1):
                        h = 2 * a + r
                        col = 128 * (tq % 4) + 64 * r
                        for tk in range(tq + 1):
                            nc.tensor.matmul(
                                psh[:, col:col + 64],
                                attset(a, r)[tk][:, 128 * tq:128 * (tq + 1)],
                                v_sb[tk][:, 64 * h:64 * (h + 1)],
                                start=(tk == 0), stop=(tk == tq),
                            )
                return psh

            # DRAM y viewed as [128 part, tq, col] for column-strip DMAs
            yr = y[:, :].rearrange("(t p) c -> p t c", p=128)

            def av_evict(a, tqs, psh, engine):
                # contiguous copy: psum half -> pair tile tq-chunks
                dst = ya[a][:, 128 * tqs[0]:128 * tqs[0] + 512]
                if engine == "act":
                    nc.scalar.copy(dst, psh[:])
                else:
                    nc.vector.tensor_scalar(dst, psh[:], 0.0, None,
                                            AluOpType.add)

            def y_strip_dma(a):
                # ship pair a's 128 output columns for all 1024 rows
                nc.sync.dma_start(
                    out=yr[:, :, 128 * a:128 * (a + 1)],
                    in_=ya[a][:].rearrange("p (t c) -> p t c", t=TT))

            # ---------- v-phase tail + pair-0 projection, interleaved ----------
            pools3 = ((psA, "psA"), (psP, "psP"), (psB, "psB"))
            for i, g in enumerate(vgroups[8:]):
                v_group(g, *pools3[i % 3])
                if i % 2 == 1:
                    proj_group(0, i // 2)

            # ---------- software-pipelined stages ----------
            # QK(a) runs a full stage ahead of AV(a): evictions of QK(a)
            # have a whole stage of PE work to drain under.  proj(a+1) is
            # fine-interleaved with QK(a); pair 5's QK is spread over the
            # last two stages so the final stage is mostly AV.
            #   stream: [proj(1) + QK(0)], then for a in 0..4:
            #           [QK(a+1) + proj(a+2) (+QK(5) slices)] [AV(a)],
            #           finally [AV(5) + finale evict/DMA].
            def qk_proj_stage(qa, pa, extra=()):
                for tk in range(TT):
                    if qa is not None:
                        qk_tk(qa, 0, tk)
                        qk_tk(qa, 1, tk)
                    if pa is not None and tk % 2 == 1:
                        proj_group(pa, tk // 2)
                for (ea, etk) in extra:
                    qk_tk(ea, 0, etk)
                    qk_tk(ea, 1, etk)

            qk_proj_stage(0, 1)
            for a in range(5):
                qa = a + 1 if a + 1 < 5 else None
                pa = a + 2 if a + 2 <= 5 else None
                if a == 3:
                    # stage 3: QK(4) + proj(5), then pair-5 tk0-2
                    qk_proj_stage(4, 5, extra=[(5, 0), (5, 1), (5, 2)])
                elif a == 4:
                    # stage 4: rest of pair-5 QK
                    qk_proj_stage(None, None,
                                  extra=[(5, tk) for tk in range(3, TT)])
                else:
                    qk_proj_stage(a + 1, a + 2)
                psh1 = av_part(a, (0, 1, 2, 3))
                av_evict(a, (0, 1, 2, 3), psh1, "act")
                psh2 = av_part(a, (4, 5, 6, 7))
                av_evict(a, (4, 5, 6, 7), psh2, "act")
                y_strip_dma(a)

            # ---------- finale: AV(5), quarter evict + strip DMA ----------
            # separate psum tiles per tq pair: PSUM deps are tile-granular,
            # so interleaving evicts with the next tq's accumulation on one
            # tile would serialize the PE on eviction latency
            a = NPAIR - 1
            for quarter in range(4):
                tqs = (2 * quarter, 2 * quarter + 1)
                psh = psB.tile([128, 512], F32, name="ps_av", tag="psB")
                for tq in tqs:
                    for r in (0, 1):
                        h = 2 * a + r
                        col = 128 * (tq % 2) + 64 * r
                        for tk in range(tq + 1):
                            nc.tensor.matmul(
                                psh[:, col:col + 64],
                                attset(a, r)[tk][:, 128 * tq:128 * (tq + 1)],
                                v_sb[tk][:, 64 * h:64 * (h + 1)],
                                start=(tk == 0), stop=(tk == tq),
                            )
                # evict both tq chunks, then ship this quarter's strip
                dst = ya[a][:, 256 * quarter:256 * (quarter + 1)]
                if quarter % 2 == 0:
                    nc.scalar.copy(dst, psh[:, :256])
                else:
                    nc.vector.tensor_scalar(dst, psh[:, :256], 0.0, None,
                                            AluOpType.add)
                eng = (nc.sync, nc.gpsimd)[quarter % 2]
                eng.dma_start(
                    out=yr[:, 2 * quarter:2 * quarter + 2,
                           128 * a:128 * (a + 1)],
                    in_=ya[a][:, 256 * quarter:256 * (quarter + 1)]
                    .rearrange("p (t c) -> p t c", t=2))

    nc.compile()
    return nc


def _prep_host(x, W_attn, b_attn):
    s = 1.0 / np.sqrt(np.float32(HD))
    W = np.asarray(W_attn, dtype=np.float32).copy()
    b = np.asarray(b_attn, dtype=np.float32).copy()
    W[:C] *= s
    b[:C] *= s
    # interleave q/k head pairs: [q-pair0, k-pair0, q-pair1, k-pair1, ...], v natural
    rows = []
    for a in range(NPAIR):
        rows.extend(range(128 * a, 128 * (a + 1)))          # q heads 2a, 2a+1
        rows.extend(range(C + 128 * a, C + 128 * (a + 1)))  # k heads 2a, 2a+1
    rows.extend(range(2 * C, 3 * C))                        # v natural
    W_perm = W[rows]
    b_perm = b[rows]

    wT = np.ascontiguousarray(W_perm.T.astype(np.float16))   # (C, 3C)
    bqk = np.ascontiguousarray(b_perm[:2 * C].reshape(2 * NPAIR, 128).T)  # (128, 12)
    bvb = np.ascontiguousarray(np.broadcast_to(b_perm[2 * C:], (128, C)))
    tri = (np.arange(128)[None, :] >= np.arange(128)[:, None]).astype(np.float32)
    masks = np.ones((128, 512), dtype=np.float32)
    masks[:, 0:128] = tri          # kept pieces always start at the diagonal
    xT = np.ascontiguousarray(np.asarray(x, dtype=np.float32).transpose(0, 2, 1).astype(np.float16))  # (B, C, T)
    return xT, wT, bqk, bvb, masks


def kernel(x, W_attn, b_attn):
    if "nc" not in _CACHE:
        _CACHE["nc"] = _build()
    nc = _CACHE["nc"]

    xT, wT, bqk, bvb, masks = _prep_host(x, W_attn, b_attn)
    in_maps = [
        {"xT": xT[c], "wT": wT, "bqk": bqk, "bvb": bvb, "masks": masks}
        for c in range(B)
    ]
    res = run_bass_kernel_spmd(nc, in_maps, list(range(B)))
    yout = np.empty((B, T, C), dtype=np.float32)
    for c in range(B):
        yout[c] = res.results[c]["y"].astype(np.float32)
    return yout
